# revision 9
# baseline (speedup 1.0000x reference)
"""TRN2 Bass/Tile kernel: GNN message-passing self-attention (BertSelfAttention style).

Math (per node n, head h):
    q = h @ Wq.T + bq                     (local nodes only)
    k/v = (h @ W{k,v}.T + b{k,v})[neighbor_idx]   (gather of transformed rows)
    scores = q.k / sqrt(dh) + mask[neighbor_idx]
    ctx = softmax(scores) @ v

Strategy (8 cores, SPMD, no collectives):
  - Shard destination nodes across cores (2500/core). Replicate h / weights.
  - Each core builds the full fused KV table [N, 512] bf16 in its DRAM via PE
    (host ships h pre-transposed so lhsT needs no on-device transpose), then
    per 128-node tile gathers all 2048 neighbor rows with one dma_gather.
  - Per-edge work on DVE with broadcast access patterns; exp on ACT.
    1/sqrt(dh) is folded into Wq; softmax normalization folded into the final
    context scaling.
"""

import math

import ml_dtypes
import numpy as np

import concourse.bass as bass
import concourse.mybir as mybir
import concourse.tile as tile
from concourse import bacc
from concourse.bass_utils import run_bass_kernel_spmd

# Problem constants (fixed by the harness contract).
N_CORES = 8
N_NODES = 20000
H = 256  # hidden size
D = 16  # neighbors per node
NH = 8  # heads
DH = 32  # head dim
P = 128  # partitions
KVW = 2 * H  # fused K|V row width

LOCAL = N_NODES // N_CORES  # 2500
NT = (LOCAL + P - 1) // P  # 20 node tiles per core
LPAD = NT * P  # 2560

F32 = mybir.dt.float32
BF16 = mybir.dt.bfloat16
I16 = mybir.dt.int16
BF = ml_dtypes.bfloat16


def build_program(n_nodes=N_NODES, local_pad=LPAD, with_bias=False):
    """Build the SPMD single-core Bass program (identical across cores)."""
    nt = local_pad // P
    kv_blocks = [
        (b0, min(512, n_nodes - b0)) for b0 in range(0, n_nodes, 512)
    ]

    nc = bacc.Bacc("TRN2", target_bir_lowering=False, debug=False)

    hT = nc.dram_tensor("hT", [H, n_nodes], BF16, kind="ExternalInput")
    hTl = nc.dram_tensor("hTl", [H, local_pad], BF16, kind="ExternalInput")
    wkv = nc.dram_tensor("wkv", [H, KVW], BF16, kind="ExternalInput")
    wq = nc.dram_tensor("wq", [H, H], BF16, kind="ExternalInput")
    idx = nc.dram_tensor("idx", [P, nt, D], mybir.dt.int32, kind="ExternalInput")
    maskg = nc.dram_tensor("maskg", [P, nt, D], F32, kind="ExternalInput")
    if with_bias:
        bkv = nc.dram_tensor("bkv", [1, KVW], BF16, kind="ExternalInput")
        bqs = nc.dram_tensor("bqs", [1, H], BF16, kind="ExternalInput")
    out = nc.dram_tensor("out", [local_pad, H], F32, kind="ExternalOutput")
    kvtab = nc.dram_tensor("kvtab", [n_nodes, KVW], BF16)

    # (bacc's compile pipeline auto-inserts the GPSIMD library load that
    # dma_gather needs, and lowers multi-wait sync_info for walrus.)
    with tile.TileContext(nc) as tc:
        with (
            tc.tile_pool(name="weights", bufs=1) as wpool,
            tc.tile_pool(name="ht", bufs=3) as htpool,
            tc.tile_pool(name="kvstage", bufs=3) as kvspool,
            tc.tile_pool(name="psum", bufs=4, space="PSUM") as pspool,
            tc.tile_pool(name="persist", bufs=1) as persist,
            tc.tile_pool(name="gath", bufs=3) as gpool,
            tc.tile_pool(name="prod", bufs=2) as prodpool,
            tc.tile_pool(name="small", bufs=3) as smpool,
            tc.tile_pool(name="ctx", bufs=3) as ctxpool,
        ):
            # ---- weights to SBUF (chunked over the 256-row contraction) ----
            wkv_t = wpool.tile([P, 2, KVW], BF16)
            nc.sync.dma_start(wkv_t[:, 0, :], wkv[0:P, :])
            nc.sync.dma_start(wkv_t[:, 1, :], wkv[P:H, :])
            wq_t = wpool.tile([P, 2, H], BF16)
            nc.sync.dma_start(wq_t[:, 0, :], wq[0:P, :])
            nc.sync.dma_start(wq_t[:, 1, :], wq[P:H, :])
            if with_bias:
                ones_t = wpool.tile([1, P], BF16)
                nc.vector.memset(ones_t[:], 1.0)
                bkv_t = wpool.tile([1, KVW], BF16)
                nc.sync.dma_start(bkv_t[:], bkv[:])
                bqs_t = wpool.tile([1, H], BF16)
                nc.sync.dma_start(bqs_t[:], bqs[:])

            # ---- persistent per-core state ----
            q_all = persist.tile([P, nt, H], BF16)
            idx_all = persist.tile([P, nt, D], mybir.dt.int32)
            nc.sync.dma_start(idx_all[:], idx[:])
            mask_all = persist.tile([P, nt, D], F32)
            nc.sync.dma_start(mask_all[:], maskg[:])

            # ---- phase 1a: local Q = hTl.T @ (Wq*scale).T (+ bq*scale) ----
            for b in range(local_pad // 512):
                ht_t = htpool.tile([P, 2, 512], BF16, tag="ht")
                nc.sync.dma_start(ht_t[:, 0, :], hTl[0:P, b * 512 : (b + 1) * 512])
                nc.sync.dma_start(ht_t[:, 1, :], hTl[P:H, b * 512 : (b + 1) * 512])
                for s in range(4):
                    t_glob = b * 4 + s
                    pq = pspool.tile([P, H], F32, tag="psq")
                    nc.tensor.matmul(
                        pq[:],
                        ht_t[:, 0, s * P : (s + 1) * P],
                        wq_t[:, 0, :],
                        start=True,
                        stop=False,
                    )
                    nc.tensor.matmul(
                        pq[:],
                        ht_t[:, 1, s * P : (s + 1) * P],
                        wq_t[:, 1, :],
                        start=False,
                        stop=not with_bias,
                    )
                    if with_bias:
                        nc.tensor.matmul(
                            pq[:],
                            ones_t[:],
                            bqs_t[:],
                            start=False,
                            stop=True,
                        )
                    nc.scalar.copy(q_all[:, t_glob, :], pq[:])

            # ---- phase 1b: full KV table = hT.T @ [Wk.T | Wv.T] (+ [bk|bv]) ----
            for b, (c0, bw) in enumerate(kv_blocks):
                nsub = (bw + P - 1) // P
                ht_t = htpool.tile([P, 2, 512], BF16, tag="ht")
                nc.sync.dma_start(ht_t[:, 0, :bw], hT[0:P, c0 : c0 + bw])
                nc.sync.dma_start(ht_t[:, 1, :bw], hT[P:H, c0 : c0 + bw])
                kv_stage = kvspool.tile([P, 4, KVW], BF16)
                for s in range(nsub):
                    sw = min(P, bw - s * P)
                    pkv = pspool.tile([P, KVW], F32, tag="pskv")
                    nc.tensor.matmul(
                        pkv[:sw, :],
                        ht_t[:, 0, s * P : s * P + sw],
                        wkv_t[:, 0, :],
                        start=True,
                        stop=False,
                    )
                    nc.tensor.matmul(
                        pkv[:sw, :],
                        ht_t[:, 1, s * P : s * P + sw],
                        wkv_t[:, 1, :],
                        start=False,
                        stop=not with_bias,
                    )
                    if with_bias:
                        nc.tensor.matmul(
                            pkv[:sw, :],
                            ones_t[:, :sw],
                            bkv_t[:],
                            start=False,
                            stop=True,
                        )
                    # alternate ACT/DVE for the cast copies
                    if s % 2 == 0:
                        nc.scalar.copy(kv_stage[:sw, s, :], pkv[:sw, :])
                    else:
                        nc.vector.tensor_copy(kv_stage[:sw, s, :], pkv[:sw, :])
                if bw == 512:
                    nc.sync.dma_start(
                        kvtab[c0 : c0 + bw, :].rearrange("(s p) e -> p s e", p=P),
                        kv_stage[:],
                    )
                else:
                    for s in range(nsub):
                        sw = min(P, bw - s * P)
                        nc.sync.dma_start(
                            kvtab[c0 + s * P : c0 + s * P + sw, :],
                            kv_stage[:sw, s, :],
                        )

            # ---- phase 2: gather + attention per 128-node tile ----
            for t in range(nt):
                kvg = gpool.tile([P, D, KVW], BF16)
                # SWDGE indirect gather: one row per partition per call (the
                # only HW-verified pattern on this image; dma_gather's ucode
                # library is unavailable, multi-offset indirect misbehaves).
                for d in range(D):
                    nc.gpsimd.indirect_dma_start(
                        out=kvg[:, d, :],
                        out_offset=None,
                        in_=kvtab[:],
                        in_offset=bass.IndirectOffsetOnAxis(
                            ap=idx_all[:, t, d : d + 1], axis=0
                        ),
                    )
                k_view = kvg[:, :, 0:H].rearrange("p d (nh dh) -> p d nh dh", nh=NH)
                v_view = kvg[:, :, H:KVW].rearrange("p d (nh dh) -> p d nh dh", nh=NH)
                q_view = (
                    q_all[:, t, :]
                    .rearrange("p (nh dh) -> p nh dh", nh=NH)
                    .unsqueeze(1)
                    .broadcast_to([P, D, NH, DH])
                )

                # scores[p, d, h] = sum_c q*k  (scale pre-folded into Wq)
                mk = prodpool.tile([P, D, NH, DH], BF16, tag="mk")
                nc.vector.tensor_tensor(
                    out=mk[:], in0=k_view, in1=q_view, op=mybir.AluOpType.mult
                )
                scores = smpool.tile([P, D, NH], F32, tag="scores")
                nc.vector.tensor_reduce(
                    out=scores[:],
                    in_=mk[:],
                    axis=mybir.AxisListType.X,
                    op=mybir.AluOpType.add,
                )
                # + gathered attention mask (broadcast over heads)
                scores_m = smpool.tile([P, D, NH], F32, tag="scores_m")
                nc.vector.tensor_tensor(
                    out=scores_m[:],
                    in0=scores[:],
                    in1=mask_all[:, t, :].unsqueeze(2).broadcast_to([P, D, NH]),
                    op=mybir.AluOpType.add,
                )
                # softmax over d (unnormalized; 1/sum folded into final scale)
                smax = smpool.tile([P, NH], F32, tag="smax")
                nc.vector.tensor_reduce(
                    out=smax[:],
                    in_=scores_m[:].transpose([0, 2, 1]),
                    axis=mybir.AxisListType.X,
                    op=mybir.AluOpType.max,
                )
                s2 = smpool.tile([P, D, NH], F32, tag="s2")
                nc.vector.tensor_tensor(
                    out=s2[:],
                    in0=scores_m[:],
                    in1=smax[:].unsqueeze(1).broadcast_to([P, D, NH]),
                    op=mybir.AluOpType.subtract,
                )
                pexp = smpool.tile([P, D, NH], BF16, tag="pexp")
                nc.scalar.activation(
                    pexp[:], s2[:], mybir.ActivationFunctionType.Exp
                )
                sumexp = smpool.tile([P, NH], F32, tag="sumexp")
                nc.vector.tensor_reduce(
                    out=sumexp[:],
                    in_=pexp[:].transpose([0, 2, 1]),
                    axis=mybir.AxisListType.X,
                    op=mybir.AluOpType.add,
                )
                rsum = smpool.tile([P, NH], F32, tag="rsum")
                nc.vector.reciprocal(rsum[:], sumexp[:])

                # ctx[p, h, c] = (sum_d pexp * v) * rsum
                mv = prodpool.tile([P, D, NH, DH], BF16, tag="mv")
                nc.vector.tensor_tensor(
                    out=mv[:],
                    in0=v_view,
                    in1=pexp[:].unsqueeze(3).broadcast_to([P, D, NH, DH]),
                    op=mybir.AluOpType.mult,
                )
                ctx_un = ctxpool.tile([P, NH, DH], F32, tag="ctx_un")
                nc.vector.tensor_reduce(
                    out=ctx_un[:],
                    in_=mv[:].transpose([0, 2, 3, 1]),
                    axis=mybir.AxisListType.X,
                    op=mybir.AluOpType.add,
                )
                ctx_f = ctxpool.tile([P, NH, DH], F32, tag="ctx_f")
                nc.vector.tensor_tensor(
                    out=ctx_f[:],
                    in0=ctx_un[:],
                    in1=rsum[:].unsqueeze(2).broadcast_to([P, NH, DH]),
                    op=mybir.AluOpType.mult,
                )
                nc.sync.dma_start(
                    out[t * P : (t + 1) * P, :],
                    ctx_f[:].rearrange("p nh dh -> p (nh dh)"),
                )

    nc.finalize()
    return nc


def prepare_inputs(
    h, attention_mask, neighbor_idx, Wq, bq, Wk, bk, Wv, bv,
    n_nodes=N_NODES, n_cores=N_CORES, local_pad=LPAD,
):
    """Host-side sharding / layout prep. Returns (in_maps, with_bias)."""
    local = n_nodes // n_cores
    nt = local_pad // P
    scale = np.float32(1.0 / math.sqrt(DH))

    h = np.asarray(h, dtype=np.float32)
    attention_mask = np.asarray(attention_mask, dtype=np.float32)
    neighbor_idx = np.asarray(neighbor_idx)
    Wq = np.asarray(Wq, dtype=np.float32)
    Wk = np.asarray(Wk, dtype=np.float32)
    Wv = np.asarray(Wv, dtype=np.float32)
    bq = np.asarray(bq, dtype=np.float32)
    bk = np.asarray(bk, dtype=np.float32)
    bv = np.asarray(bv, dtype=np.float32)

    with_bias = bool(np.any(bq) or np.any(bk) or np.any(bv))

    hT = np.ascontiguousarray(h.T).astype(BF)  # [H, N]
    wkv = np.ascontiguousarray(np.concatenate([Wk.T, Wv.T], axis=1)).astype(BF)
    wq = np.ascontiguousarray((Wq * scale).T).astype(BF)
    bkv = np.concatenate([bk, bv])[None, :].astype(BF)
    bqs = (bq * scale)[None, :].astype(BF)

    in_maps = []
    for c in range(n_cores):
        lo = c * local
        nb = np.zeros((local_pad, D), dtype=np.int64)
        nb[:local] = neighbor_idx[lo : lo + local]
        # per-tile [128, D] neighbor ids: idx[p, t, d] = nb[t*128+p, d]
        idxc = np.ascontiguousarray(
            nb.reshape(nt, P, D).transpose(1, 0, 2)
        ).astype(np.int32)

        mg = attention_mask[nb]  # [local_pad, D]
        mg = np.ascontiguousarray(
            mg.reshape(nt, P, D).transpose(1, 0, 2)
        ).astype(np.float32)

        hTl = np.zeros((H, local_pad), dtype=BF)
        hTl[:, :local] = hT[:, lo : lo + local]

        m = dict(hT=hT, hTl=hTl, wkv=wkv, wq=wq, idx=idxc, maskg=mg)
        if with_bias:
            m["bkv"] = bkv
            m["bqs"] = bqs
        in_maps.append(m)
    return in_maps, with_bias


_PROGRAM_CACHE = {}


def _get_program(with_bias):
    key = with_bias
    if key not in _PROGRAM_CACHE:
        _PROGRAM_CACHE[key] = build_program(with_bias=with_bias)
    return _PROGRAM_CACHE[key]


def kernel(h, attention_mask, neighbor_idx, Wq, bq, Wk, bk, Wv, bv, **run_kwargs):
    in_maps, with_bias = prepare_inputs(
        h, attention_mask, neighbor_idx, Wq, bq, Wk, bk, Wv, bv
    )
    nc = _get_program(with_bias)
    res = run_bass_kernel_spmd(nc, in_maps, list(range(N_CORES)), **run_kwargs)
    out = np.concatenate(
        [np.asarray(res.results[c]["out"])[:LOCAL] for c in range(N_CORES)], axis=0
    )
    result = out.astype(np.float32)
    if run_kwargs:
        return result, res
    return result
